# revision 12
# baseline (speedup 1.0000x reference)
"""Trainium2 Bass kernel for Bottleneck+DynamicConv (B=16,C=256,H=W=64,E=4).

Data-parallel over batch: 8 NeuronCores x 2 samples each. Both 3x3 convs run
as 1D Winograd F(2,3) along H (direct in W): for each tile-row pair the four
B^T row-combinations T[u] are built on the vector engine (all +-1 coeffs,
fp16 2x-mode tensor_tensor ops), the PE contracts U[u,dx] @ T[u] (24 matmuls
of 512 free per strip-o instead of direct conv's 36), psum M[u] is evacuated
by the scalar engine as fp16, and the A^T combination (+-1) runs on the
vector engine. This cuts PE work by 1/3 vs direct fp16 convolution while
staying fp16 end to end (rel err ~1e-3; fp8 points measurably exceed the
2e-2 gate in winograd space, so none are used).

Per (sample, conv, o): 4 strips of 8 tile-rows; psum tile [128, 4u, 512]
(4 banks), two in flight. Conv1 weights are G-transformed on the host; for
conv2 the expert bank is mixed in direct space (stt with routing-gate AP
scalars) and u1 = 0.5(w0+w1+w2) / u2 = 0.5(w0-w1+w2) are built on-device;
u0/u3 alias the mixed ky0/ky2 blocks directly. Routing pools y through the
SiLU epilogue's accum_out, so no separate image reduction is needed. T
halves and kern prep are emitted ahead of the consuming strips so the PE
stream stays dense across the conv1(s0)->conv1(s1)->conv2(s0)->conv2(s1)
sequence.
"""

from contextlib import ExitStack

import numpy as np

import concourse.bacc as bacc
import concourse.bass as bass
import concourse.mybir as mybir
from concourse import tile
from concourse.bass_utils import run_bass_kernel_spmd

B, C, H, W, E = 16, 256, 64, 64, 4
KH = KW = 3
EPS = 1e-5
NCORES = 8
S = B // NCORES           # samples per core = 2
CT = C // 128             # channel tiles = 2
PD = W + 2                # padded width/height = 66
PF = PD * PD              # padded flat pixels per channel tile = 4356
HWF = H * W               # 4096
NU = 4                    # winograd points per tile-row pair
NDX = 3                   # direct column taps
NSTRIP = 4                # strips per (sample, conv, o); 8 tile-rows each
TPS = 8                   # tile-rows per strip
NN = TPS * W              # matmul free dim = 512
THALF = NU * 16 * PD      # T half tile cols = 4224
W1COLS = CT * NU * NDX * CT * 128   # 6144
KDCOLS = KH * CT * NDX * CT * 128   # 4608 direct blocks (ky, o, dx, ci)
KYB = KDCOLS // 3                   # 1536 = one ky block group
F16 = mybir.dt.float16
F32 = mybir.dt.float32
NPF16 = np.float16
Alu = mybir.AluOpType

TRACE = False
LAST_EXEC_NS = None
ACT_FUNC = mybir.ActivationFunctionType.Silu

_prog_cache = {}


def _build_program():
    nc = bacc.Bacc(
        "TRN2", target_bir_lowering=False, debug=False,
        enable_asserts=False, num_devices=NCORES)

    xpad_d = nc.dram_tensor("xpad", [S, CT, 128, PF], F16, kind="ExternalInput")
    w1u_d = nc.dram_tensor("w1u", [128, W1COLS], F16, kind="ExternalInput")
    bank_d = nc.dram_tensor("bank", [128, E * KDCOLS], F16, kind="ExternalInput")
    wr_d = nc.dram_tensor("wrt", [128, CT * E], F32, kind="ExternalInput")
    br_d = nc.dram_tensor("brb", [128, E], F32, kind="ExternalInput")
    b1_d = nc.dram_tensor("b1sb", [128, CT], F32, kind="ExternalInput")
    b2_d = nc.dram_tensor("b2sb", [128, CT], F32, kind="ExternalInput")
    out_d = nc.dram_tensor("out", [S, CT, 128, HWF], F16, kind="ExternalOutput")

    with tile.TileContext(nc) as tc, ExitStack() as ctx:
        const = ctx.enter_context(tc.tile_pool(name="const", bufs=1))
        xp_pool = ctx.enter_context(tc.tile_pool(name="xp", bufs=2))
        yp_pool = ctx.enter_context(tc.tile_pool(name="yp", bufs=2))
        t_pool = ctx.enter_context(tc.tile_pool(name="tp", bufs=2))
        kd_pool = ctx.enter_context(tc.tile_pool(name="kd", bufs=2))
        ku_pool = ctx.enter_context(tc.tile_pool(name="ku", bufs=4))
        m_pool = ctx.enter_context(tc.tile_pool(name="m16", bufs=3))
        o_pool = ctx.enter_context(tc.tile_pool(name="ost", bufs=2))
        small = ctx.enter_context(tc.tile_pool(name="small", bufs=2))
        ps_pool = ctx.enter_context(tc.tile_pool(name="ps", bufs=2, space="PSUM"))

        # HAM warmup: burn the NEFF-preamble DMA window on dummy matmuls so
        # the PE clock-gate is fully open when real work starts.
        dummy_t = const.tile([128, 128], F16)
        nc.gpsimd.memset(dummy_t[:], 0.0)
        warm_ps = ps_pool.tile([128, NU, NN], F32, tag="M")
        for _ in range(56):
            nc.tensor.matmul(
                warm_ps[:, 0:1, 0:128], dummy_t[:], dummy_t[:],
                start=True, stop=True)

        # constants + conv1 winograd weights (o=0 half first: it gates the
        # first psum group)
        w1u_t = const.tile([128, W1COLS], F16)
        HC = W1COLS // 2
        b1_t = const.tile([128, CT], F32)
        wr_t = const.tile([128, CT * E], F32)
        br_t = const.tile([128, E], F32)
        b2_t = const.tile([128, CT], F32)
        ones_t = const.tile([128, 128], F32)
        nc.vector.memset(ones_t[:], 1.0)
        half_t = const.tile([128, 1], F32)
        nc.vector.memset(half_t[:], 0.5)
        bank_t = const.tile([128, E * KDCOLS], F16)

        # input DMA in consumption order: s0 rows 0..33 both ci (gates the
        # first T ops), w1u second half, s0 rows 34..65, then s1, then the
        # expert bank (needed only after conv1(s0)'s routing), split rings.
        # consumption-ordered small pieces over two rings (sync: ci0 +
        # u0/u3 weights; gpsimd: ci1 + u1/u2 weights); the scalar ring is
        # kept free for the mix/evac/silu work. ~0.7 KB(/partition)/us per
        # ring, so no piece may block a sooner-needed one.
        R1, R2, R3 = 18 * PD, 34 * PD, 50 * PD
        xpts = [xp_pool.tile([128, CT * PF], F16, tag="xp", name=f"xp{i}")
                for i in range(S)]

        def xput(s_, ci, lo, hi, eng):
            eng.dma_start(xpts[s_][:, ci * PF + lo:ci * PF + hi],
                          xpad_d.ap()[s_, ci][:, lo:hi])

        def wput(o, u, eng):
            lo = (o * NU + u) * NDX * CT * 128
            hi = lo + NDX * CT * 128
            eng.dma_start(w1u_t[:, lo:hi], w1u_d.ap()[:, lo:hi])

        nc.sync.dma_start(b1_t[:], b1_d.ap())
        xput(0, 0, 0, R1, nc.sync)
        xput(0, 1, 0, R1, nc.gpsimd)
        wput(0, 0, nc.sync)
        wput(0, 1, nc.gpsimd)
        wput(0, 3, nc.sync)
        wput(0, 2, nc.gpsimd)
        xput(0, 0, R1, R2, nc.sync)
        xput(0, 1, R1, R2, nc.gpsimd)
        wput(1, 0, nc.sync)
        wput(1, 1, nc.gpsimd)
        wput(1, 3, nc.sync)
        wput(1, 2, nc.gpsimd)
        xput(0, 0, R2, R3, nc.sync)
        xput(0, 1, R2, R3, nc.gpsimd)
        xput(0, 0, R3, PF, nc.sync)
        xput(0, 1, R3, PF, nc.gpsimd)
        nc.sync.dma_start(wr_t[:], wr_d.ap())
        nc.sync.dma_start(br_t[:], br_d.ap())
        nc.sync.dma_start(b2_t[:], b2_d.ap())
        xput(1, 0, 0, PF, nc.sync)
        xput(1, 1, 0, PF, nc.gpsimd)
        for e, eng in ((0, nc.sync), (2, nc.gpsimd), (1, nc.sync),
                       (3, nc.gpsimd)):
            eng.dma_start(bank_t[:, e * KDCOLS:(e + 1) * KDCOLS],
                          bank_d.ap()[:, e * KDCOLS:(e + 1) * KDCOLS])

        # y tiles + pad-ring zeroing upfront (disjoint from the interior the
        # SiLU epilogues write, so no false ordering)
        yts, paccs = [], []
        for s in range(S):
            yt = yp_pool.tile([128, CT * PF], F16, tag="yp")
            yts.append(yt)
            for ci in range(CT):
                yv = yt[:, ci * PF:(ci + 1) * PF].rearrange(
                    "p (h w) -> p h w", h=PD)
                nc.gpsimd.memset(yv[:, 0:1, :], 0.0)
                nc.gpsimd.memset(yv[:, PD - 1:PD, :], 0.0)
                nc.gpsimd.memset(yv[:, :, 0:1], 0.0)
                nc.gpsimd.memset(yv[:, :, PD - 1:PD], 0.0)
            pacc = small.tile([128, CT * NSTRIP], F32, tag=f"pacc{s}", bufs=1)
            paccs.append(pacc)

        def w1col(u, dx, o, ci):
            blk = ((o * NU + u) * NDX + dx) * CT + ci
            return w1u_t[:, blk * 128:(blk + 1) * 128]

        def make_kcol(kd_t, u1_t, u2_t):
            def kcol(u, dx, o, ci):
                if u == 0 or u == 3:
                    ky = 0 if u == 0 else 2
                    blk = ((ky * CT + o) * NDX + dx) * CT + ci
                    return kd_t[:, blk * 128:(blk + 1) * 128]
                t = u1_t if u == 1 else u2_t
                blk = (o * NDX + dx) * CT + ci
                return t[:, blk * 128:(blk + 1) * 128]
            return kcol

        def emit_T(src_tile, ci, h0, dst, t_off=0, nrow=16):
            """B^T row transform for nrow tile-rows at tile-row h0+t_off:
            dst[u][t_off:t_off+nrow] from padded rows 2*(h0+t_off+t)+k."""
            xr = src_tile[:, ci * PF:(ci + 1) * PF].rearrange(
                "p (t f w) -> p t f w", t=33, f=2)
            dv = [dst[:, u * 16 * PD + t_off * PD:
                      u * 16 * PD + (t_off + nrow) * PD].rearrange(
                "p (t f w) -> p t f w", t=nrow, f=1) for u in range(NU)]
            b0 = h0 + t_off
            d0 = xr[:, b0:b0 + nrow, 0:1, :]
            d1 = xr[:, b0:b0 + nrow, 1:2, :]
            d2 = xr[:, b0 + 1:b0 + 1 + nrow, 0:1, :]
            d3 = xr[:, b0 + 1:b0 + 1 + nrow, 1:2, :]
            nc.vector.tensor_sub(dv[0], d0, d2)
            nc.vector.tensor_add(dv[1], d1, d2)
            nc.vector.tensor_sub(dv[2], d2, d1)
            nc.vector.tensor_sub(dv[3], d1, d3)

        def alloc_T():
            return {ci: t_pool.tile([128, THALF], F16, tag=f"T{ci}",
                                    name=f"Th{ci}") for ci in range(CT)}

        def emit_T_half(src_tile, half, quarters=False):
            tiles = alloc_T()
            if quarters:
                for q in range(2):
                    for ci in range(CT):
                        emit_T(src_tile, ci, half * 16, tiles[ci],
                               t_off=q * 8, nrow=8)
            else:
                for ci in range(CT):
                    emit_T(src_tile, ci, half * 16, tiles[ci])
            return tiles

        kds = {}

        def emit_routing(s):
            """pooled -> gates (sigmoid via silu(x)/x: no act-table swap)
            -> kd allocated + expert-0 term."""
            pacc = paccs[s]
            r_t = small.tile([128, E], F32, tag="r")
            psr = ps_pool.tile([128, NU, NN], F32, tag="M")
            for o in range(CT):
                pooled = small.tile([128, 2], F32, tag="pooled")
                nc.vector.tensor_add(
                    pooled[:], pacc[:, o * NSTRIP:o * NSTRIP + 2],
                    pacc[:, o * NSTRIP + 2:o * NSTRIP + 4])
                nc.vector.tensor_add(
                    pooled[:, 0:1], pooled[:, 0:1], pooled[:, 1:2])
                pbc = small.tile([128, 128], F32, tag="pbc")
                nc.vector.tensor_scalar_mul(pbc[:], ones_t[:], pooled[:, 0:1])
                nc.tensor.matmul(
                    psr[:, 0:1, 0:E], pbc[:], wr_t[:, o * E:(o + 1) * E],
                    start=(o == 0), stop=(o == CT - 1))
            logits = small.tile([128, E], F32, tag="logits")
            nc.vector.tensor_add(
                logits[:].rearrange("p (f e) -> p f e", f=1),
                psr[:, 0:1, 0:E],
                br_t[:].rearrange("p (f e) -> p f e", f=1))
            silu_l = small.tile([128, E], F32, tag="silu_l")
            nc.scalar.activation(silu_l[:], logits[:], ACT_FUNC)
            rec = small.tile([128, E], F32, tag="rec")
            nc.vector.reciprocal(rec[:], logits[:])
            nc.vector.tensor_mul(r_t[:], silu_l[:], rec[:])
            kd_t = kd_pool.tile([128, KDCOLS], F16, tag="kd")
            nc.vector.tensor_scalar_mul(
                kd_t[:], bank_t[:, 0:KDCOLS], r_t[:, 0:1])
            kds[s] = (kd_t, r_t)

        def mix_piece(s, e, g):
            """kd[g] += bank_e[g] * r_e: scalar-engine multiply (Copy act
            with the gate as scale AP), DVE 2x add."""
            kd_t, r_t = kds[s]
            lo = e * KDCOLS + g * KYB
            tmp = small.tile([128, KYB], F16, tag="ktmp", bufs=2,
                             name=f"mixtmp{e}{g}")
            nc.scalar.activation(
                tmp[:], bank_t[:, lo:lo + KYB],
                mybir.ActivationFunctionType.Copy, scale=r_t[:, e:e + 1])
            nc.vector.tensor_add(
                kd_t[:, g * KYB:(g + 1) * KYB],
                kd_t[:, g * KYB:(g + 1) * KYB], tmp[:])

        def emit_kern_transform(s):
            # u1 = 0.5(ky0+ky1+ky2), u2 = 0.5(ky0-ky1+ky2)
            kd_t, _ = kds[s]
            p_t = small.tile([128, KYB], F16, tag="ktmp", bufs=2)
            h_t = small.tile([128, KYB], F16, tag="ktmp", bufs=2)
            nc.vector.tensor_add(p_t[:], kd_t[:, 0:KYB], kd_t[:, 2 * KYB:])
            nc.vector.tensor_scalar_mul(
                h_t[:], kd_t[:, KYB:2 * KYB], half_t[:, 0:1])
            u1_t = ku_pool.tile([128, KYB], F16, tag="ku")
            u2_t = ku_pool.tile([128, KYB], F16, tag="ku")
            nc.vector.scalar_tensor_tensor(
                u1_t[:], p_t[:], half_t[:, 0:1], h_t[:], Alu.mult, Alu.add)
            nc.vector.scalar_tensor_tensor(
                u2_t[:], p_t[:], half_t[:, 0:1], h_t[:], Alu.mult,
                Alu.subtract)
            kcols[s] = make_kcol(kd_t, u1_t, u2_t)

        def emit_conv(s, conv, hooks, pre_h0=None):
            """One conv layer for sample s. conv=0: x->y (silu+pool accum);
            conv=1: y->out (silu+residual+DMA). hooks[(strip, o)] thunks are
            emitted after that psum group's drain, spreading routing / mix /
            T-prefetch work evenly so no engine queue sees a burst."""
            src = xpts[s] if conv == 0 else yts[s]
            bias_t = b1_t if conv == 0 else b2_t
            first = pre_h0 is None
            t_half = {0: pre_h0 if not first
                      else emit_T_half(src, 0, quarters=True)}
            if first:
                th1 = alloc_T()
                t_half[1] = th1
                for i, (q, ci) in enumerate(
                        ((0, 0), (0, 1), (8, 0), (8, 1))):
                    hooks.setdefault((i // 2, i % 2), []).append(
                        (lambda q=q, ci=ci:
                         emit_T(src, ci, 16, th1[ci], q, 8)))
            else:
                t_half[1] = emit_T_half(src, 1)

            for th in hooks.get((-1, 0), []):
                th()
            for strip in range(NSTRIP):
                half = strip // 2
                t0 = (strip % 2) * TPS
                wcol = w1col if conv == 0 else kcols[s]
                last = conv == 1 and s == S - 1 and strip == NSTRIP - 1
                for o in range(CT):
                    m16 = m_pool.tile([128, NU * NN], F16, tag="m16")
                    m2 = m16.rearrange("p (u n) -> p u n", u=NU)
                    splits = 2 if (last and o == CT - 1) else 1
                    nt = TPS // splits
                    tvs = [[t_half[half][ci][
                        :, u * 16 * PD:(u + 1) * 16 * PD].rearrange(
                        "p (t w) -> p t w", t=16) for ci in range(CT)]
                        for u in range(NU)]
                    for hf in range(splits):
                        ps = ps_pool.tile([128, NU, nt * W], F32, tag="M",
                                          name="psh")
                        tlo = t0 + hf * nt
                        nlo, nhi = hf * nt * W, (hf + 1) * nt * W
                        for u in (0, 3, 1, 2):
                            idx = 0
                            for dx in range(NDX):
                                for ci in range(CT):
                                    nc.tensor.matmul(
                                        ps[:, u:u + 1, :],
                                        wcol(u, dx, o, ci),
                                        tvs[u][ci][:, tlo:tlo + nt,
                                                   dx:dx + W],
                                        start=(idx == 0), stop=(idx == 5))
                                    idx += 1
                        nc.scalar.copy(m2[:, :, nlo:nhi], ps[:])
                        mv = [m16[:, u * NN + nlo:u * NN + nhi].rearrange(
                            "p (t f w) -> p t f w", t=nt, f=1)
                            for u in range(NU)]
                        # A^T combine in place: a0 -> u0 on gpsimd, a1 ->
                        # u1 on vector (parallel engines; the framework
                        # orders the u1 WAR). The final groups stay on the
                        # vector engine to keep the drain tail short.
                        tail_g = last or (conv == 1 and s == S - 1
                                          and strip == NSTRIP - 2)
                        e0 = nc.vector if tail_g else nc.gpsimd
                        e0.tensor_add(mv[0], mv[0], mv[1])
                        e0.tensor_add(mv[0], mv[0], mv[2])
                        nc.vector.tensor_sub(mv[1], mv[1], mv[2])
                        nc.vector.tensor_sub(mv[1], mv[1], mv[3])
                        av = m16.rearrange(
                            "p (a t w) -> p t a w", a=NU,
                            t=TPS)[:, hf * nt:hf * nt + nt, 0:2, :]
                        r0 = strip * 2 * TPS + hf * 2 * nt
                        if conv == 0:
                            yr = yts[s][:, o * PF + PD:
                                        o * PF + PD + 64 * PD].rearrange(
                                "p (t f w) -> p t f w", t=32, f=2)
                            nc.scalar.activation(
                                yr[:, r0 // 2:r0 // 2 + nt, :, 1:1 + W],
                                av, ACT_FUNC, bias=bias_t[:, o:o + 1],
                                accum_out=paccs[s][:, o * NSTRIP + strip:
                                                   o * NSTRIP + strip + 1])
                        else:
                            ost = o_pool.tile(
                                [128, 2 * nt * W], F16, tag="ost")
                            ov = ost.rearrange(
                                "p (t f w) -> p t f w", t=nt, f=2)
                            nc.scalar.activation(
                                ov, av, ACT_FUNC, bias=bias_t[:, o:o + 1])
                            xr = xpts[s][:, o * PF + (1 + r0) * PD:
                                         o * PF + (1 + r0 + 2 * nt) * PD]\
                                .rearrange("p (t w) -> p t w", t=2 * nt)
                            orow = ost.rearrange("p (t w) -> p t w", t=2 * nt)
                            nc.vector.tensor_add(
                                orow, orow, xr[:, :, 1:1 + W])
                            eng = [nc.gpsimd, nc.sync][(strip + o + hf) % 2]
                            eng.dma_start(
                                out_d.ap()[s, o][:, r0 * W:
                                                 (r0 + 2 * nt) * W],
                                ost[:])
                    for th in hooks.get((strip, o), []):
                        th()

        def mix_hooks(s, t_src):
            """Spread routing + 9 mix pieces + T prefetch + transform over
            the host conv's 8 half-strip hook slots. The prefetch T tiles
            are allocated inside the hook so the tile-ring rotation stays
            aligned with consumption order."""
            def pre_t0():
                th = alloc_T()
                nxt[f"T{s}"] = th
                emit_T(t_src, 0, 0, th[0])

            def pre_t1():
                emit_T(t_src, 1, 0, nxt[f"T{s}"][1])

            return {
                (-1, 0): [lambda: emit_routing(s)],
                (0, 0): [lambda: mix_piece(s, 1, 0)],
                (0, 1): [lambda: mix_piece(s, 1, 1),
                         lambda: mix_piece(s, 1, 2)],
                (1, 0): [lambda: mix_piece(s, 2, 0),
                         lambda: mix_piece(s, 2, 1), pre_t0],
                (1, 1): [lambda: mix_piece(s, 2, 2),
                         lambda: mix_piece(s, 3, 0), pre_t1],
                (2, 0): [lambda: mix_piece(s, 3, 1),
                         lambda: mix_piece(s, 3, 2)],
                (2, 1): [lambda: emit_kern_transform(s)],
            }

        nxt = {}
        kcols = {}

        def pre_c1s1_a():
            nxt["c1s1"] = alloc_T()
            emit_T(xpts[1], 0, 0, nxt["c1s1"][0])

        emit_conv(0, 0, {(1, 1): [pre_c1s1_a],
                         (2, 0): [lambda: emit_T(
                             xpts[1], 1, 0, nxt["c1s1"][1])]})
        emit_conv(1, 0, mix_hooks(0, yts[0]), pre_h0=nxt["c1s1"])
        emit_conv(0, 1, mix_hooks(1, yts[1]), pre_h0=nxt["T0"])
        emit_conv(1, 1, {}, pre_h0=nxt["T1"])

    nc.compile()
    return nc


def _get_program():
    if "nc" not in _prog_cache:
        _prog_cache["nc"] = _build_program()
    return _prog_cache["nc"]


# F(2,3) weight transform G (winograd rows u from conv taps ky)
_G = np.array([[1, 0, 0], [.5, .5, .5], [.5, -.5, .5], [0, 0, 1]], np.float32)


def kernel(x, w1, bn1_g, bn1_b, bn1_m, bn1_v, wr, br, w_e,
           bn2_g, bn2_b, bn2_m, bn2_v):
    global LAST_EXEC_NS
    f32 = np.float32
    x = np.ascontiguousarray(np.asarray(x, f32))
    w1 = np.asarray(w1, f32)
    wr = np.asarray(wr, f32)
    br = np.asarray(br, f32)
    w_e = np.asarray(w_e, f32)

    s1 = np.asarray(bn1_g, f32) / np.sqrt(np.asarray(bn1_v, f32) + EPS)
    b1 = np.asarray(bn1_b, f32) - np.asarray(bn1_m, f32) * s1
    s2 = np.asarray(bn2_g, f32) / np.sqrt(np.asarray(bn2_v, f32) + EPS)
    b2 = np.asarray(bn2_b, f32) - np.asarray(bn2_m, f32) * s2

    # conv1 weights: BN1 scale fold, winograd G-transform over ky, lhsT
    # layout [cin_p, (o, u, dx, ci, cout)]
    w1f = (w1 * s1[:, None, None, None]).reshape(CT, 128, CT, 128, KH, KW)
    w1uf = np.einsum('uk,apbqkd->qaudbp', _G, w1f)
    w1u = np.ascontiguousarray(w1uf.reshape(128, W1COLS)).astype(NPF16)

    # expert bank in direct space, ky-major: [cin_p, e, (ky, o, dx, ci, cout)]
    wef = (w_e.reshape(E, C, C, KH, KW)
           * s2[None, :, None, None, None]).reshape(E, CT, 128, CT, 128,
                                                    KH, KW)
    bank = np.ascontiguousarray(
        wef.transpose(4, 0, 5, 1, 6, 3, 2).reshape(128, E * KDCOLS)
    ).astype(NPF16)

    wrt = np.ascontiguousarray(
        (wr / HWF).reshape(E, CT, 128).transpose(2, 1, 0).reshape(128, CT * E))
    brb = np.ascontiguousarray(np.broadcast_to(br, (128, E)))
    b1sb = np.ascontiguousarray(b1.reshape(CT, 128).T)
    b2sb = np.ascontiguousarray(b2.reshape(CT, 128).T)

    pad = np.zeros((B, CT, 128, PD, PD), f32)
    pad[:, :, :, 1:H + 1, 1:W + 1] = x.reshape(B, CT, 128, H, W)
    xpad = np.ascontiguousarray(pad.reshape(B, CT, 128, PF).astype(NPF16))

    nc = _get_program()
    in_maps = []
    for c in range(NCORES):
        sl = slice(S * c, S * (c + 1))
        in_maps.append({
            "xpad": np.ascontiguousarray(xpad[sl]),
            "w1u": w1u, "bank": bank, "wrt": wrt, "brb": brb,
            "b1sb": b1sb, "b2sb": b2sb,
        })

    res = run_bass_kernel_spmd(
        nc, in_maps, core_ids=list(range(NCORES)), trace=TRACE)
    LAST_EXEC_NS = res.exec_time_ns

    out = np.empty((B, C, H, W), f32)
    for c in range(NCORES):
        out[S * c:S * (c + 1)] = res.results[c]["out"].reshape(
            S, C, H, W).astype(f32)
    return out


if __name__ == "__main__":
    rng = np.random.default_rng(0)
    f32 = np.float32
    ins = {
        "x": rng.standard_normal((B, C, H, W), f32),
        "w1": rng.standard_normal((C, C, KH, KW), f32) * 0.05,
        "bn1_g": np.ones(C, f32), "bn1_b": np.zeros(C, f32),
        "bn1_m": rng.standard_normal(C, f32) * 0.05,
        "bn1_v": np.abs(rng.standard_normal(C, f32) * 0.05) + 1.0,
        "wr": rng.standard_normal((E, C), f32) * 0.05,
        "br": np.zeros(E, f32),
        "w_e": rng.standard_normal((E, C * C * KH * KW), f32) * 0.05,
        "bn2_g": np.ones(C, f32), "bn2_b": np.zeros(C, f32),
        "bn2_m": rng.standard_normal(C, f32) * 0.05,
        "bn2_v": np.abs(rng.standard_normal(C, f32) * 0.05) + 1.0,
    }
    o = kernel(**ins)
    print(o.shape, o.dtype)


# revision 13
# speedup vs baseline: 1.1601x; 1.1601x over previous
"""Trainium2 Bass kernel for Bottleneck+DynamicConv (B=16,C=256,H=W=64,E=4).

Data-parallel over batch: 8 NeuronCores x 2 samples each. Both 3x3 convs run
as 1D Winograd F(2,3) along H (direct in W): for each tile-row pair the four
B^T row-combinations T[u] are built on the vector engine (all +-1 coeffs,
fp16 2x-mode tensor_tensor ops), the PE contracts U[u,dx] @ T[u] (24 matmuls
of 512 free per strip-o instead of direct conv's 36), psum M[u] is evacuated
by the scalar engine as fp16, and the A^T combination (+-1) runs on the
vector engine. This cuts PE work by 1/3 vs direct fp16 convolution while
staying fp16 end to end (rel err ~1e-3; fp8 points measurably exceed the
2e-2 gate in winograd space, so none are used).

Per (sample, conv, o): 4 strips of 8 tile-rows; psum tile [128, 4u, 512]
(4 banks), two in flight. Conv1 weights are G-transformed on the host; for
conv2 the expert bank is mixed in direct space (stt with routing-gate AP
scalars) and u1 = 0.5(w0+w1+w2) / u2 = 0.5(w0-w1+w2) are built on-device;
u0/u3 alias the mixed ky0/ky2 blocks directly. Routing pools y through the
SiLU epilogue's accum_out, so no separate image reduction is needed. T
halves and kern prep are emitted ahead of the consuming strips so the PE
stream stays dense across the conv1(s0)->conv1(s1)->conv2(s0)->conv2(s1)
sequence.
"""

from contextlib import ExitStack

import numpy as np

import concourse.bacc as bacc
import concourse.bass as bass
import concourse.mybir as mybir
from concourse import tile
from concourse.bass_utils import run_bass_kernel_spmd

B, C, H, W, E = 16, 256, 64, 64, 4
KH = KW = 3
EPS = 1e-5
NCORES = 8
S = B // NCORES           # samples per core = 2
CT = C // 128             # channel tiles = 2
PD = W + 2                # padded width/height = 66
PF = PD * PD              # padded flat pixels per channel tile = 4356
HWF = H * W               # 4096
NU = 4                    # winograd points per tile-row pair
NDX = 3                   # direct column taps
NSTRIP = 4                # strips per (sample, conv, o); 8 tile-rows each
TPS = 8                   # tile-rows per strip
NN = TPS * W              # matmul free dim = 512
THALF = NU * 16 * PD      # T half tile cols = 4224
W1COLS = CT * NU * NDX * CT * 128   # 6144
KDCOLS = KH * CT * NDX * CT * 128   # 4608 direct blocks (ky, o, dx, ci)
KYB = KDCOLS // 3                   # 1536 = one ky block group
F16 = mybir.dt.float16
F32 = mybir.dt.float32
NPF16 = np.float16
Alu = mybir.AluOpType

TRACE = False
LAST_EXEC_NS = None
ACT_FUNC = mybir.ActivationFunctionType.Silu

_prog_cache = {}


def _build_program():
    nc = bacc.Bacc(
        "TRN2", target_bir_lowering=False, debug=False,
        enable_asserts=False, num_devices=NCORES)

    xpad_d = nc.dram_tensor("xpad", [S, CT, 128, PF], F16, kind="ExternalInput")
    w1u_d = nc.dram_tensor("w1u", [128, W1COLS], F16, kind="ExternalInput")
    bank_d = nc.dram_tensor("bank", [128, E * KDCOLS], F16, kind="ExternalInput")
    wr_d = nc.dram_tensor("wrt", [128, CT * E], F32, kind="ExternalInput")
    br_d = nc.dram_tensor("brb", [128, E], F32, kind="ExternalInput")
    b1_d = nc.dram_tensor("b1sb", [128, CT], F32, kind="ExternalInput")
    b2_d = nc.dram_tensor("b2sb", [128, CT], F32, kind="ExternalInput")
    out_d = nc.dram_tensor("out", [S, CT, 128, HWF], F16, kind="ExternalOutput")

    with tile.TileContext(nc) as tc, ExitStack() as ctx:
        const = ctx.enter_context(tc.tile_pool(name="const", bufs=1))
        xp_pool = ctx.enter_context(tc.tile_pool(name="xp", bufs=2))
        yp_pool = ctx.enter_context(tc.tile_pool(name="yp", bufs=2))
        t_pool = ctx.enter_context(tc.tile_pool(name="tp", bufs=2))
        kd_pool = ctx.enter_context(tc.tile_pool(name="kd", bufs=2))
        ku_pool = ctx.enter_context(tc.tile_pool(name="ku", bufs=4))
        m_pool = ctx.enter_context(tc.tile_pool(name="m16", bufs=3))
        o_pool = ctx.enter_context(tc.tile_pool(name="ost", bufs=2))
        small = ctx.enter_context(tc.tile_pool(name="small", bufs=2))
        ps_pool = ctx.enter_context(tc.tile_pool(name="ps", bufs=2, space="PSUM"))

        # HAM warmup: burn the NEFF-preamble DMA window on dummy matmuls so
        # the PE clock-gate is fully open when real work starts.
        dummy_t = const.tile([128, 128], F16)
        nc.gpsimd.memset(dummy_t[:], 0.0)
        warm_ps = ps_pool.tile([128, NU, NN], F32, tag="M")
        for _ in range(56):
            nc.tensor.matmul(
                warm_ps[:, 0:1, 0:128], dummy_t[:], dummy_t[:],
                start=True, stop=True)

        # constants + conv1 winograd weights (o=0 half first: it gates the
        # first psum group)
        w1u_t = const.tile([128, W1COLS], F16)
        HC = W1COLS // 2
        b1_t = const.tile([128, CT], F32)
        wr_t = const.tile([128, CT * E], F32)
        br_t = const.tile([128, E], F32)
        b2_t = const.tile([128, CT], F32)
        ones_t = const.tile([128, 128], F32)
        nc.vector.memset(ones_t[:], 1.0)
        half_t = const.tile([128, 1], F32)
        nc.vector.memset(half_t[:], 0.5)
        bank_t = const.tile([128, E * KDCOLS], F16)

        # input DMA in consumption order: s0 rows 0..33 both ci (gates the
        # first T ops), w1u second half, s0 rows 34..65, then s1, then the
        # expert bank (needed only after conv1(s0)'s routing), split rings.
        # consumption-ordered small pieces over two rings (sync: ci0 +
        # u0/u3 weights; gpsimd: ci1 + u1/u2 weights); the scalar ring is
        # kept free for the mix/evac/silu work. ~0.7 KB(/partition)/us per
        # ring, so no piece may block a sooner-needed one.
        R1, R2, R3 = 18 * PD, 34 * PD, 50 * PD
        xpts = [xp_pool.tile([128, CT * PF], F16, tag="xp", name=f"xp{i}")
                for i in range(S)]

        def xput(s_, ci, lo, hi, eng):
            eng.dma_start(xpts[s_][:, ci * PF + lo:ci * PF + hi],
                          xpad_d.ap()[s_, ci][:, lo:hi])

        def wput(o, u, eng):
            lo = (o * NU + u) * NDX * CT * 128
            hi = lo + NDX * CT * 128
            eng.dma_start(w1u_t[:, lo:hi], w1u_d.ap()[:, lo:hi])

        nc.sync.dma_start(b1_t[:], b1_d.ap())
        xput(0, 0, 0, R1, nc.sync)
        xput(0, 1, 0, R1, nc.gpsimd)
        wput(0, 0, nc.sync)
        wput(0, 1, nc.gpsimd)
        wput(0, 3, nc.sync)
        wput(0, 2, nc.gpsimd)
        xput(0, 0, R1, R2, nc.sync)
        xput(0, 1, R1, R2, nc.gpsimd)
        wput(1, 0, nc.sync)
        wput(1, 1, nc.gpsimd)
        wput(1, 3, nc.sync)
        wput(1, 2, nc.gpsimd)
        xput(0, 0, R2, R3, nc.sync)
        xput(0, 1, R2, R3, nc.gpsimd)
        xput(0, 0, R3, PF, nc.sync)
        xput(0, 1, R3, PF, nc.gpsimd)
        nc.sync.dma_start(wr_t[:], wr_d.ap())
        nc.sync.dma_start(br_t[:], br_d.ap())
        nc.sync.dma_start(b2_t[:], b2_d.ap())
        xput(1, 0, 0, PF, nc.sync)
        xput(1, 1, 0, PF, nc.gpsimd)
        for e, eng in ((0, nc.sync), (2, nc.gpsimd), (1, nc.sync),
                       (3, nc.gpsimd)):
            eng.dma_start(bank_t[:, e * KDCOLS:(e + 1) * KDCOLS],
                          bank_d.ap()[:, e * KDCOLS:(e + 1) * KDCOLS])

        # y tiles + pad-ring zeroing upfront (disjoint from the interior the
        # SiLU epilogues write, so no false ordering)
        yts, paccs = [], []
        for s in range(S):
            yt = yp_pool.tile([128, CT * PF], F16, tag="yp")
            yts.append(yt)
            for ci in range(CT):
                yv = yt[:, ci * PF:(ci + 1) * PF].rearrange(
                    "p (h w) -> p h w", h=PD)
                nc.gpsimd.memset(yv[:, 0:1, :], 0.0)
                nc.gpsimd.memset(yv[:, PD - 1:PD, :], 0.0)
                nc.gpsimd.memset(yv[:, :, 0:1], 0.0)
                nc.gpsimd.memset(yv[:, :, PD - 1:PD], 0.0)
            pacc = small.tile([128, CT * NSTRIP], F32, tag=f"pacc{s}", bufs=1)
            paccs.append(pacc)

        def w1col(u, dx, o, ci):
            blk = ((o * NU + u) * NDX + dx) * CT + ci
            return w1u_t[:, blk * 128:(blk + 1) * 128]

        def make_kcol(kd_t, u1_t, u2_t):
            def kcol(u, dx, o, ci):
                if u == 0 or u == 3:
                    ky = 0 if u == 0 else 2
                    blk = ((ky * CT + o) * NDX + dx) * CT + ci
                    return kd_t[:, blk * 128:(blk + 1) * 128]
                t = u1_t if u == 1 else u2_t
                blk = (o * NDX + dx) * CT + ci
                return t[:, blk * 128:(blk + 1) * 128]
            return kcol

        def emit_T(src_tile, ci, h0, dst, t_off=0, nrow=16):
            """B^T row transform for nrow tile-rows at tile-row h0+t_off:
            dst[u][t_off:t_off+nrow] from padded rows 2*(h0+t_off+t)+k."""
            xr = src_tile[:, ci * PF:(ci + 1) * PF].rearrange(
                "p (t f w) -> p t f w", t=33, f=2)
            dv = [dst[:, u * 16 * PD + t_off * PD:
                      u * 16 * PD + (t_off + nrow) * PD].rearrange(
                "p (t f w) -> p t f w", t=nrow, f=1) for u in range(NU)]
            b0 = h0 + t_off
            d0 = xr[:, b0:b0 + nrow, 0:1, :]
            d1 = xr[:, b0:b0 + nrow, 1:2, :]
            d2 = xr[:, b0 + 1:b0 + 1 + nrow, 0:1, :]
            d3 = xr[:, b0 + 1:b0 + 1 + nrow, 1:2, :]
            nc.vector.tensor_sub(dv[0], d0, d2)
            nc.vector.tensor_add(dv[1], d1, d2)
            nc.vector.tensor_sub(dv[2], d2, d1)
            nc.vector.tensor_sub(dv[3], d1, d3)

        def alloc_T():
            return {ci: t_pool.tile([128, THALF], F16, tag=f"T{ci}",
                                    name=f"Th{ci}") for ci in range(CT)}

        def emit_T_half(src_tile, half, quarters=False):
            tiles = alloc_T()
            if quarters:
                for q in range(2):
                    for ci in range(CT):
                        emit_T(src_tile, ci, half * 16, tiles[ci],
                               t_off=q * 8, nrow=8)
            else:
                for ci in range(CT):
                    emit_T(src_tile, ci, half * 16, tiles[ci])
            return tiles

        kds = {}

        def emit_routing(s):
            """pooled -> gates (sigmoid via silu(x)/x: no act-table swap)
            -> kd allocated + expert-0 term."""
            pacc = paccs[s]
            r_t = small.tile([128, E], F32, tag="r")
            psr = ps_pool.tile([128, NU, NN], F32, tag="M")
            for o in range(CT):
                pooled = small.tile([128, 2], F32, tag="pooled")
                nc.vector.tensor_add(
                    pooled[:], pacc[:, o * NSTRIP:o * NSTRIP + 2],
                    pacc[:, o * NSTRIP + 2:o * NSTRIP + 4])
                nc.vector.tensor_add(
                    pooled[:, 0:1], pooled[:, 0:1], pooled[:, 1:2])
                pbc = small.tile([128, 128], F32, tag="pbc")
                nc.vector.tensor_scalar_mul(pbc[:], ones_t[:], pooled[:, 0:1])
                nc.tensor.matmul(
                    psr[:, 0:1, 0:E], pbc[:], wr_t[:, o * E:(o + 1) * E],
                    start=(o == 0), stop=(o == CT - 1))
            logits = small.tile([128, E], F32, tag="logits")
            nc.vector.tensor_add(
                logits[:].rearrange("p (f e) -> p f e", f=1),
                psr[:, 0:1, 0:E],
                br_t[:].rearrange("p (f e) -> p f e", f=1))
            silu_l = small.tile([128, E], F32, tag="silu_l")
            nc.scalar.activation(silu_l[:], logits[:], ACT_FUNC)
            rec = small.tile([128, E], F32, tag="rec")
            nc.vector.reciprocal(rec[:], logits[:])
            nc.vector.tensor_mul(r_t[:], silu_l[:], rec[:])
            kd_t = kd_pool.tile([128, KDCOLS], F16, tag="kd")
            nc.vector.tensor_scalar_mul(
                kd_t[:], bank_t[:, 0:KDCOLS], r_t[:, 0:1])
            kds[s] = (kd_t, r_t)

        def mix_piece(s, e, g):
            """kd[g] += bank_e[g] * r_e: scalar-engine multiply (Copy act
            with the gate as scale AP), DVE 2x add."""
            kd_t, r_t = kds[s]
            lo = e * KDCOLS + g * KYB
            tmp = small.tile([128, KYB], F16, tag="ktmp", bufs=2,
                             name=f"mixtmp{e}{g}")
            nc.scalar.activation(
                tmp[:], bank_t[:, lo:lo + KYB],
                mybir.ActivationFunctionType.Copy, scale=r_t[:, e:e + 1])
            nc.vector.tensor_add(
                kd_t[:, g * KYB:(g + 1) * KYB],
                kd_t[:, g * KYB:(g + 1) * KYB], tmp[:])

        def emit_kern_transform(s):
            # u1 = 0.5(ky0+ky1+ky2), u2 = 0.5(ky0-ky1+ky2)
            kd_t, _ = kds[s]
            p_t = small.tile([128, KYB], F16, tag="ktmp", bufs=2)
            h_t = small.tile([128, KYB], F16, tag="ktmp", bufs=2)
            nc.vector.tensor_add(p_t[:], kd_t[:, 0:KYB], kd_t[:, 2 * KYB:])
            nc.vector.tensor_scalar_mul(
                h_t[:], kd_t[:, KYB:2 * KYB], half_t[:, 0:1])
            u1_t = ku_pool.tile([128, KYB], F16, tag="ku")
            u2_t = ku_pool.tile([128, KYB], F16, tag="ku")
            nc.vector.scalar_tensor_tensor(
                u1_t[:], p_t[:], half_t[:, 0:1], h_t[:], Alu.mult, Alu.add)
            nc.vector.scalar_tensor_tensor(
                u2_t[:], p_t[:], half_t[:, 0:1], h_t[:], Alu.mult,
                Alu.subtract)
            kcols[s] = make_kcol(kd_t, u1_t, u2_t)

        def emit_conv(s, conv, hooks, pre_h0=None):
            """One conv layer for sample s. conv=0: x->y (silu+pool accum);
            conv=1: y->out (silu+residual+DMA). hooks[(strip, o)] thunks are
            emitted after that psum group's drain, spreading routing / mix /
            T-prefetch work evenly so no engine queue sees a burst."""
            src = xpts[s] if conv == 0 else yts[s]
            bias_t = b1_t if conv == 0 else b2_t
            first = pre_h0 is None
            t_half = {0: pre_h0 if not first
                      else emit_T_half(src, 0, quarters=True)}
            if first:
                th1 = alloc_T()
                t_half[1] = th1
                for i, (q, ci) in enumerate(
                        ((0, 0), (0, 1), (8, 0), (8, 1))):
                    hooks.setdefault((i // 2, i % 2), []).append(
                        (lambda q=q, ci=ci:
                         emit_T(src, ci, 16, th1[ci], q, 8)))
            else:
                t_half[1] = emit_T_half(src, 1)

            for th in hooks.get((-1, 0), []):
                th()
            for strip in range(NSTRIP):
                half = strip // 2
                t0 = (strip % 2) * TPS
                wcol = w1col if conv == 0 else kcols[s]
                last = conv == 1 and s == S - 1 and strip == NSTRIP - 1
                for o in range(CT):
                    m16 = m_pool.tile([128, NU * NN], F16, tag="m16")
                    m2 = m16.rearrange("p (u n) -> p u n", u=NU)
                    splits = 2 if (last and o == CT - 1) else 1
                    nt = TPS // splits
                    tvs = [[t_half[half][ci][
                        :, u * 16 * PD:(u + 1) * 16 * PD].rearrange(
                        "p (t w) -> p t w", t=16) for ci in range(CT)]
                        for u in range(NU)]
                    for hf in range(splits):
                        ps = ps_pool.tile([128, NU, nt * W], F32, tag="M",
                                          name="psh")
                        tlo = t0 + hf * nt
                        nlo, nhi = hf * nt * W, (hf + 1) * nt * W
                        for u in (0, 3, 1, 2):
                            idx = 0
                            for dx in range(NDX):
                                for ci in range(CT):
                                    nc.tensor.matmul(
                                        ps[:, u:u + 1, :],
                                        wcol(u, dx, o, ci),
                                        tvs[u][ci][:, tlo:tlo + nt,
                                                   dx:dx + W],
                                        start=(idx == 0), stop=(idx == 5))
                                    idx += 1
                        nc.scalar.copy(m2[:, :, nlo:nhi], ps[:])
                        mv = [m16[:, u * NN + nlo:u * NN + nhi].rearrange(
                            "p (t f w) -> p t f w", t=nt, f=1)
                            for u in range(NU)]
                        # A^T combine in place: u0 slot <- a0, u1 <- a1
                        nc.vector.tensor_add(mv[0], mv[0], mv[1])
                        nc.vector.tensor_add(mv[0], mv[0], mv[2])
                        nc.vector.tensor_sub(mv[1], mv[1], mv[2])
                        nc.vector.tensor_sub(mv[1], mv[1], mv[3])
                        av = m16.rearrange(
                            "p (a t w) -> p t a w", a=NU,
                            t=TPS)[:, hf * nt:hf * nt + nt, 0:2, :]
                        r0 = strip * 2 * TPS + hf * 2 * nt
                        if conv == 0:
                            yr = yts[s][:, o * PF + PD:
                                        o * PF + PD + 64 * PD].rearrange(
                                "p (t f w) -> p t f w", t=32, f=2)
                            nc.scalar.activation(
                                yr[:, r0 // 2:r0 // 2 + nt, :, 1:1 + W],
                                av, ACT_FUNC, bias=bias_t[:, o:o + 1],
                                accum_out=paccs[s][:, o * NSTRIP + strip:
                                                   o * NSTRIP + strip + 1])
                        else:
                            ost = o_pool.tile(
                                [128, 2 * nt * W], F16, tag="ost")
                            ov = ost.rearrange(
                                "p (t f w) -> p t f w", t=nt, f=2)
                            nc.scalar.activation(
                                ov, av, ACT_FUNC, bias=bias_t[:, o:o + 1])
                            xr = xpts[s][:, o * PF + (1 + r0) * PD:
                                         o * PF + (1 + r0 + 2 * nt) * PD]\
                                .rearrange("p (t w) -> p t w", t=2 * nt)
                            orow = ost.rearrange("p (t w) -> p t w", t=2 * nt)
                            nc.vector.tensor_add(
                                orow, orow, xr[:, :, 1:1 + W])
                            eng = [nc.gpsimd, nc.sync][(strip + o + hf) % 2]
                            eng.dma_start(
                                out_d.ap()[s, o][:, r0 * W:
                                                 (r0 + 2 * nt) * W],
                                ost[:])
                    for th in hooks.get((strip, o), []):
                        th()

        def mix_hooks(s, t_src):
            """Spread routing + 9 mix pieces + T prefetch + transform over
            the host conv's 8 half-strip hook slots. The prefetch T tiles
            are allocated inside the hook so the tile-ring rotation stays
            aligned with consumption order."""
            def pre_t0():
                th = alloc_T()
                nxt[f"T{s}"] = th
                emit_T(t_src, 0, 0, th[0])

            def pre_t1():
                emit_T(t_src, 1, 0, nxt[f"T{s}"][1])

            return {
                (-1, 0): [lambda: emit_routing(s)],
                (0, 0): [lambda: mix_piece(s, 1, 0)],
                (0, 1): [lambda: mix_piece(s, 1, 1),
                         lambda: mix_piece(s, 1, 2)],
                (1, 0): [lambda: mix_piece(s, 2, 0),
                         lambda: mix_piece(s, 2, 1), pre_t0],
                (1, 1): [lambda: mix_piece(s, 2, 2),
                         lambda: mix_piece(s, 3, 0), pre_t1],
                (2, 0): [lambda: mix_piece(s, 3, 1),
                         lambda: mix_piece(s, 3, 2)],
                (2, 1): [lambda: emit_kern_transform(s)],
            }

        nxt = {}
        kcols = {}

        def pre_c1s1_a():
            nxt["c1s1"] = alloc_T()
            emit_T(xpts[1], 0, 0, nxt["c1s1"][0])

        emit_conv(0, 0, {(1, 1): [pre_c1s1_a],
                         (2, 0): [lambda: emit_T(
                             xpts[1], 1, 0, nxt["c1s1"][1])]})
        emit_conv(1, 0, mix_hooks(0, yts[0]), pre_h0=nxt["c1s1"])
        emit_conv(0, 1, mix_hooks(1, yts[1]), pre_h0=nxt["T0"])
        emit_conv(1, 1, {}, pre_h0=nxt["T1"])

    nc.compile()
    return nc


def _get_program():
    if "nc" not in _prog_cache:
        _prog_cache["nc"] = _build_program()
    return _prog_cache["nc"]


# F(2,3) weight transform G (winograd rows u from conv taps ky)
_G = np.array([[1, 0, 0], [.5, .5, .5], [.5, -.5, .5], [0, 0, 1]], np.float32)


def kernel(x, w1, bn1_g, bn1_b, bn1_m, bn1_v, wr, br, w_e,
           bn2_g, bn2_b, bn2_m, bn2_v):
    global LAST_EXEC_NS
    f32 = np.float32
    x = np.ascontiguousarray(np.asarray(x, f32))
    w1 = np.asarray(w1, f32)
    wr = np.asarray(wr, f32)
    br = np.asarray(br, f32)
    w_e = np.asarray(w_e, f32)

    s1 = np.asarray(bn1_g, f32) / np.sqrt(np.asarray(bn1_v, f32) + EPS)
    b1 = np.asarray(bn1_b, f32) - np.asarray(bn1_m, f32) * s1
    s2 = np.asarray(bn2_g, f32) / np.sqrt(np.asarray(bn2_v, f32) + EPS)
    b2 = np.asarray(bn2_b, f32) - np.asarray(bn2_m, f32) * s2

    # conv1 weights: BN1 scale fold, winograd G-transform over ky, lhsT
    # layout [cin_p, (o, u, dx, ci, cout)]
    w1f = (w1 * s1[:, None, None, None]).reshape(CT, 128, CT, 128, KH, KW)
    w1uf = np.einsum('uk,apbqkd->qaudbp', _G, w1f)
    w1u = np.ascontiguousarray(w1uf.reshape(128, W1COLS)).astype(NPF16)

    # expert bank in direct space, ky-major: [cin_p, e, (ky, o, dx, ci, cout)]
    wef = (w_e.reshape(E, C, C, KH, KW)
           * s2[None, :, None, None, None]).reshape(E, CT, 128, CT, 128,
                                                    KH, KW)
    bank = np.ascontiguousarray(
        wef.transpose(4, 0, 5, 1, 6, 3, 2).reshape(128, E * KDCOLS)
    ).astype(NPF16)

    wrt = np.ascontiguousarray(
        (wr / HWF).reshape(E, CT, 128).transpose(2, 1, 0).reshape(128, CT * E))
    brb = np.ascontiguousarray(np.broadcast_to(br, (128, E)))
    b1sb = np.ascontiguousarray(b1.reshape(CT, 128).T)
    b2sb = np.ascontiguousarray(b2.reshape(CT, 128).T)

    pad = np.zeros((B, CT, 128, PD, PD), f32)
    pad[:, :, :, 1:H + 1, 1:W + 1] = x.reshape(B, CT, 128, H, W)
    xpad = np.ascontiguousarray(pad.reshape(B, CT, 128, PF).astype(NPF16))

    nc = _get_program()
    in_maps = []
    for c in range(NCORES):
        sl = slice(S * c, S * (c + 1))
        in_maps.append({
            "xpad": np.ascontiguousarray(xpad[sl]),
            "w1u": w1u, "bank": bank, "wrt": wrt, "brb": brb,
            "b1sb": b1sb, "b2sb": b2sb,
        })

    res = run_bass_kernel_spmd(
        nc, in_maps, core_ids=list(range(NCORES)), trace=TRACE)
    LAST_EXEC_NS = res.exec_time_ns

    out = np.empty((B, C, H, W), f32)
    for c in range(NCORES):
        out[S * c:S * (c + 1)] = res.results[c]["out"].reshape(
            S, C, H, W).astype(f32)
    return out


if __name__ == "__main__":
    rng = np.random.default_rng(0)
    f32 = np.float32
    ins = {
        "x": rng.standard_normal((B, C, H, W), f32),
        "w1": rng.standard_normal((C, C, KH, KW), f32) * 0.05,
        "bn1_g": np.ones(C, f32), "bn1_b": np.zeros(C, f32),
        "bn1_m": rng.standard_normal(C, f32) * 0.05,
        "bn1_v": np.abs(rng.standard_normal(C, f32) * 0.05) + 1.0,
        "wr": rng.standard_normal((E, C), f32) * 0.05,
        "br": np.zeros(E, f32),
        "w_e": rng.standard_normal((E, C * C * KH * KW), f32) * 0.05,
        "bn2_g": np.ones(C, f32), "bn2_b": np.zeros(C, f32),
        "bn2_m": rng.standard_normal(C, f32) * 0.05,
        "bn2_v": np.abs(rng.standard_normal(C, f32) * 0.05) + 1.0,
    }
    o = kernel(**ins)
    print(o.shape, o.dtype)
